# revision 1
# baseline (speedup 1.0000x reference)
"""BotRGCN on 8 Trainium2 NeuronCores (Bass/Tile, SPMD).

Strategy (per sharding hint): nodes row-sharded across 8 cores, edges
partitioned by destination node. Per RGCN layer, each core computes the
per-relation transformed features h_r = x @ W_r for its node shard
(node-major, fp16); the shards are AllGathered so every core holds the
full h table, then each core aggregates messages for its destination
shard with an indexed DMA row-gather followed by one-hot S-matrix
matmuls on the tensor engine that perform the segment-mean (1/cnt
folded into S). S blocks are generated on-device with a single DVE op
per block (iota == dst7) * invc, so no S matrices move over HBM.
Root/weight matmuls run feat-major from SBUF-resident x^T tiles; layer
outputs are PE-transposed back to feat-major for the next layer.

Self-contained: hardcodes problem shapes; the host side shards inputs,
builds gather indices + per-edge (dst7, 1/cnt) tables, compiles one
SPMD Bass program and runs it on cores 0-7.
"""
import sys

import numpy as np

for _p in ("/opt/trn_rl_repo",):
    if _p not in sys.path:
        sys.path.insert(0, _p)

import concourse.bacc as bacc
import concourse.mybir as mybir
from concourse import tile

dt = mybir.dt

NCORES = 8
SLOPE = 0.01
DEFAULT_VARIANT = "sp0q4"
CHUNK = 32768  # int16 gather-index limit per table slice


def _ceil_to(x, m):
    return ((x + m - 1) // m) * m


class Meta:
    pass


def prepare(inputs, ncores=NCORES, node_chunk=448):
    """Shard inputs, build gather indices + per-edge S-gen tables."""
    m = Meta()
    N = inputs["des"].shape[0]
    m.N = N
    m.ncores = ncores
    nsh = N // ncores
    assert nsh * ncores == N
    npad = _ceil_to(nsh, 128)
    m.nsh, m.npad = nsh, npad
    T = npad // 128
    m.ntiles = T
    m.node_chunk = node_chunk
    assert npad % node_chunk == 0
    rows = ncores * 2 * npad  # h table: [core][rel][node]
    m.rows = rows
    nch = (rows + CHUNK - 1) // CHUNK
    m.ngr = nch
    m.chunk_sizes = [min(CHUNK, rows - g * CHUNK) for g in range(nch)]

    src = np.asarray(inputs["edge_index"][0], dtype=np.int64)
    dst = np.asarray(inputs["edge_index"][1], dtype=np.int64)
    rel = np.asarray(inputs["edge_type"], dtype=np.int64)

    cnt = np.zeros((2, N), np.int64)
    for r in (0, 1):
        sel = rel == r
        cnt[r] = np.bincount(dst[sel], minlength=N)
    invc = 1.0 / np.maximum(cnt, 1).astype(np.float32)

    core_s, loc_s = src // nsh, src % nsh
    flat = core_s * (2 * npad) + rel * npad + loc_s
    g_e = flat // CHUNK
    idx16 = flat - g_e * CHUNK
    coeff = invc[rel, dst]

    core_d = dst // nsh
    locd = dst % nsh
    tile_d = locd // 128
    dst7 = locd % 128

    counts = np.zeros((ncores, T, m.ngr), np.int64)
    np.add.at(counts, (core_d, tile_d, g_e), 1)
    bud = counts.max(axis=0)
    bud = np.where(bud > 0, ((bud + 15) // 16) * 16, 0)  # 16-granular
    m.bud = bud

    # per-(t,g): column offsets in the idx stream (16-granular) and
    # block structure (ceil(B/128) blocks, last may be partial)
    m.goff = np.zeros((T, m.ngr), np.int64)
    m.gnb = np.zeros((T, m.ngr), np.int64)
    off = 0
    for t in range(T):
        for g in range(m.ngr):
            m.goff[t, g] = off
            off += bud[t, g]
            m.gnb[t, g] = -(-bud[t, g] // 128)
    m.totidx = int(off)
    m.nb_tile = m.gnb.sum(axis=1)
    m.totnb = int(m.nb_tile.sum())
    m.tile_nboff = np.zeros(T, np.int64)
    off = 0
    for t in range(T):
        m.tile_nboff[t] = off
        off += m.nb_tile[t]

    in_maps = []
    order = np.lexsort((idx16, g_e, tile_d, core_d))
    o_idx16, o_dst7, o_coeff = idx16[order], dst7[order], coeff[order]
    key = (core_d[order] * T + tile_d[order]) * m.ngr + g_e[order]
    uniq_start = np.searchsorted(key, np.arange(ncores * T * m.ngr), "left")
    uniq_end = np.searchsorted(key, np.arange(ncores * T * m.ngr), "right")

    des = np.asarray(inputs["des"], np.float32)
    tweet = np.asarray(inputs["tweet"], np.float32)
    nump = np.asarray(inputs["num_prop"], np.float32)
    catp = np.asarray(inputs["cat_prop"], np.float32)

    def shard_T(x, c, kblocks):
        xs = x[c * nsh : (c + 1) * nsh]
        out = np.zeros((kblocks * 128, npad), np.float32)
        out[: x.shape[1], :nsh] = xs.T
        return out.reshape(kblocks, 128, npad)

    def small_T(x, c, d):
        xs = x[c * nsh : (c + 1) * nsh]
        out = np.zeros((d, npad), np.float32)
        out[: x.shape[1], :nsh] = xs.T
        return out

    w = {}
    w["Wd6"] = np.ascontiguousarray(
        np.asarray(inputs["Wd"], np.float32).reshape(6, 128, 64)
    )
    w["Wt6"] = np.ascontiguousarray(
        np.asarray(inputs["Wt"], np.float32).reshape(6, 128, 64)
    )
    w["Wn"] = np.asarray(inputs["Wn"], np.float32)
    w["Wc"] = np.asarray(inputs["Wc"], np.float32)
    w["Wi2"] = np.asarray(inputs["Wi"], np.float32).reshape(2, 128, 256)
    w["W1"] = np.asarray(inputs["rel_w1"], np.float32).reshape(2, 2, 128, 256)
    w["root1"] = np.asarray(inputs["root_w1"], np.float32).reshape(2, 128, 256)
    w["W2"] = np.asarray(inputs["rel_w2"], np.float32).reshape(2, 2, 128, 256)
    w["root2"] = np.asarray(inputs["root_w2"], np.float32).reshape(2, 128, 256)
    w["Wo1"] = np.asarray(inputs["Wo1"], np.float32).reshape(2, 128, 256)
    w["Wo2"] = np.asarray(inputs["Wo2"], np.float32).reshape(2, 128, 2)
    w["bias_a"] = (
        np.concatenate([np.asarray(inputs["bd"]), np.asarray(inputs["bt"])])
        .astype(np.float32)
        .reshape(128, 1)
    )
    w["bias_b"] = (
        np.concatenate([np.asarray(inputs["bn"]), np.asarray(inputs["bc"])])
        .astype(np.float32)
        .reshape(128, 1)
    )
    w["bi_col"] = np.asarray(inputs["bi"], np.float32).reshape(2, 128).T.copy()
    w["bias1_rep"] = np.tile(
        np.asarray(inputs["bias1"], np.float32)[None, :], (128, 1)
    )
    w["bias2_rep"] = np.tile(
        np.asarray(inputs["bias2"], np.float32)[None, :], (128, 1)
    )
    w["bo1_col"] = np.asarray(inputs["bo1"], np.float32).reshape(2, 128).T.copy()
    w["bo2_rep"] = np.tile(
        np.asarray(inputs["bo2"], np.float32)[None, :], (128, 1)
    )
    w["ident"] = np.eye(128, dtype=np.float32)
    w["iota"] = np.tile(
        np.arange(128, dtype=np.float16)[None, :], (128, 1)
    )

    for c in range(ncores):
        idx_all = np.zeros((m.totidx,), np.int16)
        dstb = np.zeros((m.totnb * 128,), np.float32)
        invcb = np.full((m.totnb * 128,), 0, np.float32)
        # unmatched sentinel for padded slots: dst7 = -1 never matches iota
        dstb[:] = -1.0
        for t in range(T):
            for g in range(m.ngr):
                B = int(bud[t, g])
                if B == 0:
                    continue
                u = (c * T + t) * m.ngr + g
                s0, s1 = uniq_start[u], uniq_end[u]
                n = s1 - s0
                o = int(m.goff[t, g])
                idx_all[o : o + n] = o_idx16[s0:s1].astype(np.int16)
                boff = int(
                    (
                        m.tile_nboff[t]
                        + m.gnb[t, :g].sum()
                    )
                    * 128
                )
                e = np.arange(n)
                dstb[boff + e] = o_dst7[s0:s1].astype(np.float32)
                invcb[boff + e] = o_coeff[s0:s1].astype(np.float32)
        idx_w = np.zeros((128, m.totidx // 16), np.int16)
        for t in range(T):
            for g in range(m.ngr):
                B = int(bud[t, g])
                if B == 0:
                    continue
                o = int(m.goff[t, g])
                seg = idx_all[o : o + B].reshape(B // 16, 16).T
                idx_w[:, o // 16 : (o + B) // 16] = np.tile(seg, (8, 1))
        im = {
            "desT": shard_T(des, c, 6),
            "tweetT": shard_T(tweet, c, 6),
            "numT": small_T(nump, c, 5),
            "catT": small_T(catp, c, 3),
            "idx": idx_w,
            "dstb": np.ascontiguousarray(
                dstb.reshape(m.totnb, 128).T
            ),  # [128, totnb]
            "invcb": np.ascontiguousarray(invcb.reshape(m.totnb, 128).T),
        }
        im.update(w)
        in_maps.append(im)
    return m, in_maps


# ---------------------------------------------------------------- builder
def build(m, debug=False, repeats=1, variant="full"):
    nq = 4 if "q4" in variant else 1
    nc = bacc.Bacc(
        "TRN2",
        target_bir_lowering=False,
        debug=debug,
        enable_asserts=True,
        num_devices=m.ncores,
        num_swdge_queues=nq,
    )
    npad, T = m.npad, m.ntiles
    NCH = m.node_chunk
    NGR = m.ngr

    f32, f16, i16 = dt.float32, dt.float16, dt.int16
    ein, eout = "ExternalInput", "ExternalOutput"

    desT = nc.dram_tensor("desT", [6, 128, npad], f32, kind=ein)
    tweetT = nc.dram_tensor("tweetT", [6, 128, npad], f32, kind=ein)
    numT = nc.dram_tensor("numT", [5, npad], f32, kind=ein)
    catT = nc.dram_tensor("catT", [3, npad], f32, kind=ein)
    idx_ext = nc.dram_tensor("idx", [128, m.totidx // 16], i16, kind=ein)
    dst_ext = nc.dram_tensor("dstb", [128, m.totnb], f32, kind=ein)
    invc_ext = nc.dram_tensor("invcb", [128, m.totnb], f32, kind=ein)
    Wd6 = nc.dram_tensor("Wd6", [6, 128, 64], f32, kind=ein)
    Wt6 = nc.dram_tensor("Wt6", [6, 128, 64], f32, kind=ein)
    Wn = nc.dram_tensor("Wn", [5, 64], f32, kind=ein)
    Wc = nc.dram_tensor("Wc", [3, 64], f32, kind=ein)
    Wi2 = nc.dram_tensor("Wi2", [2, 128, 256], f32, kind=ein)
    W1 = nc.dram_tensor("W1", [2, 2, 128, 256], f32, kind=ein)
    root1 = nc.dram_tensor("root1", [2, 128, 256], f32, kind=ein)
    W2 = nc.dram_tensor("W2", [2, 2, 128, 256], f32, kind=ein)
    root2 = nc.dram_tensor("root2", [2, 128, 256], f32, kind=ein)
    Wo1 = nc.dram_tensor("Wo1", [2, 128, 256], f32, kind=ein)
    Wo2 = nc.dram_tensor("Wo2", [2, 128, 2], f32, kind=ein)
    bias_a = nc.dram_tensor("bias_a", [128, 1], f32, kind=ein)
    bias_b = nc.dram_tensor("bias_b", [128, 1], f32, kind=ein)
    bi_col = nc.dram_tensor("bi_col", [128, 2], f32, kind=ein)
    bias1_rep = nc.dram_tensor("bias1_rep", [128, 256], f32, kind=ein)
    bias2_rep = nc.dram_tensor("bias2_rep", [128, 256], f32, kind=ein)
    bo1_col = nc.dram_tensor("bo1_col", [128, 2], f32, kind=ein)
    bo2_rep = nc.dram_tensor("bo2_rep", [128, 2], f32, kind=ein)
    ident = nc.dram_tensor("ident", [128, 128], f32, kind=ein)
    iota_ext = nc.dram_tensor("iota", [128, 128], f16, kind=ein)
    out_ext = nc.dram_tensor("out", [npad, 2], f32, kind=eout)

    rp1 = nc.dram_tensor("rp1", [npad, 256], f16)
    rp2 = nc.dram_tensor("rp2", [npad, 256], f16)
    h1_loc = nc.dram_tensor("h1_loc", [2 * npad, 256], f16)
    h1_full = nc.dram_tensor("h1_full", [m.rows, 256], f16, addr_space="Shared")
    h2_loc = nc.dram_tensor("h2_loc", [2 * npad, 256], f16)
    h2_full = nc.dram_tensor("h2_full", [m.rows, 256], f16, addr_space="Shared")

    def AG(loc, full):
        if "no_ag" in variant:
            nc.sync.dma_start(full.ap()[0 : loc.shape[0], :], loc[:])
        else:
            nc.gpsimd.collective_compute(
                "AllGather",
                mybir.AluOpType.bypass,
                ins=[loc[:]],
                outs=[full[:]],
                replica_groups=[list(range(m.ncores))],
            )

    def lrelu_from(pool, dst_ap, src_ap, bias_ap, shape):
        t0 = pool.tile(shape, f32, tag="lr0", name="lr0")
        nc.scalar.activation(
            t0[:], src_ap, mybir.ActivationFunctionType.Identity, bias=bias_ap
        )
        t1 = pool.tile(shape, f32, tag="lr1", name="lr1")
        nc.vector.tensor_scalar_mul(t1[:], t0[:], SLOPE)
        nc.vector.tensor_max(dst_ap, t0[:], t1[:])

    with tile.TileContext(nc) as tc:
        with (
            tc.tile_pool(name="wpool", bufs=1) as wp,
            tc.tile_pool(name="xres", bufs=1) as xres,
        ):
            wd_sb = wp.tile([128, 6, 64], f32)
            nc.sync.dma_start(wd_sb[:], _pmaj(Wd6))
            wt_sb = wp.tile([128, 6, 64], f32)
            nc.sync.dma_start(wt_sb[:], _pmaj(Wt6))
            wn_sb = wp.tile([5, 64], f32)
            nc.sync.dma_start(wn_sb[:], Wn[:])
            wc_sb = wp.tile([3, 64], f32)
            nc.sync.dma_start(wc_sb[:], Wc[:])
            wi_sb = wp.tile([128, 2, 256], f32)
            nc.sync.dma_start(wi_sb[:], _pmaj(Wi2))
            w1_sb = wp.tile([128, 4, 256], f32)
            nc.sync.dma_start(w1_sb[:], W1.ap().rearrange("r k p m -> p (r k) m"))
            r1_sb = wp.tile([128, 2, 256], f32)
            nc.sync.dma_start(r1_sb[:], _pmaj(root1))
            w2_sb = wp.tile([128, 4, 256], f32)
            nc.sync.dma_start(w2_sb[:], W2.ap().rearrange("r k p m -> p (r k) m"))
            r2_sb = wp.tile([128, 2, 256], f32)
            nc.sync.dma_start(r2_sb[:], _pmaj(root2))
            wo1_sb = wp.tile([128, 2, 256], f32)
            nc.sync.dma_start(wo1_sb[:], _pmaj(Wo1))
            wo2_sb = wp.tile([128, 2, 2], f32)
            nc.sync.dma_start(wo2_sb[:], _pmaj(Wo2))
            ba_sb = wp.tile([128, 1], f32)
            nc.sync.dma_start(ba_sb[:], bias_a[:])
            bb_sb = wp.tile([128, 1], f32)
            nc.sync.dma_start(bb_sb[:], bias_b[:])
            bi_sb = wp.tile([128, 2], f32)
            nc.sync.dma_start(bi_sb[:], bi_col[:])
            b1_sb = wp.tile([128, 256], f32)
            nc.sync.dma_start(b1_sb[:], bias1_rep[:])
            b2_sb = wp.tile([128, 256], f32)
            nc.sync.dma_start(b2_sb[:], bias2_rep[:])
            bo1_sb = wp.tile([128, 2], f32)
            nc.sync.dma_start(bo1_sb[:], bo1_col[:])
            bo2_sb = wp.tile([128, 2], f32)
            nc.sync.dma_start(bo2_sb[:], bo2_rep[:])
            id_sb = wp.tile([128, 128], f32)
            nc.sync.dma_start(id_sb[:], ident[:])
            io_sb = wp.tile([128, 128], f16)
            nc.sync.dma_start(io_sb[:], iota_ext[:])
            idx_sb = wp.tile([128, m.totidx // 16], i16)
            nc.sync.dma_start(idx_sb[:], idx_ext[:])
            dst_sb = wp.tile([128, m.totnb], f32)
            nc.sync.dma_start(dst_sb[:], dst_ext[:])
            invc_sb = wp.tile([128, m.totnb], f32)
            nc.sync.dma_start(invc_sb[:], invc_ext[:])

            x1a = xres.tile([128, npad], f32, tag="x1a")
            x1b = xres.tile([128, npad], f32, tag="x1b")
            x2a = xres.tile([128, npad], f32, tag="x2a")
            x2b = xres.tile([128, npad], f32, tag="x2b")
            out_stage = xres.tile([128, T, 2], f32, tag="outst")

            def whole_body():
                # -------- phase 0: feature pipeline -> x1T --------
                with (
                    tc.tile_pool(name="p0", bufs=2) as p0,
                    tc.tile_pool(name="p0ps", bufs=2, space="PSUM") as p0ps,
                ):
                    for c0 in range(0, npad, NCH):
                        dsb = p0.tile([128, 6, NCH], f32, tag="des", name="dsb")
                        nc.sync.dma_start(
                            dsb[:],
                            desT.ap()[:, :, c0 : c0 + NCH].rearrange(
                                "k p n -> p k n"
                            ),
                        )
                        tsb = p0.tile([128, 6, NCH], f32, tag="tw", name="tsb")
                        nc.sync.dma_start(
                            tsb[:],
                            tweetT.ap()[:, :, c0 : c0 + NCH].rearrange(
                                "k p n -> p k n"
                            ),
                        )
                        nsb = p0.tile([5, NCH], f32, tag="np", name="nsb")
                        nc.sync.dma_start(nsb[:], numT.ap()[:, c0 : c0 + NCH])
                        csb = p0.tile([3, NCH], f32, tag="cp", name="csb")
                        nc.sync.dma_start(csb[:], catT.ap()[:, c0 : c0 + NCH])

                        ps_a = p0ps.tile([128, NCH], f32, tag="psa", name="ps_a")
                        for k in range(6):
                            nc.tensor.matmul(
                                ps_a[0:64, :],
                                wd_sb[:, k, :],
                                dsb[:, k, :],
                                start=(k == 0),
                                stop=(k == 5),
                            )
                        for k in range(6):
                            nc.tensor.matmul(
                                ps_a[64:128, :],
                                wt_sb[:, k, :],
                                tsb[:, k, :],
                                start=(k == 0),
                                stop=(k == 5),
                                tile_position=(0, 64),
                            )
                        ps_b = p0ps.tile([128, NCH], f32, tag="psb", name="ps_b")
                        nc.tensor.matmul(
                            ps_b[0:64, :], wn_sb[:], nsb[:], start=True, stop=True
                        )
                        nc.tensor.matmul(
                            ps_b[64:128, :],
                            wc_sb[:],
                            csb[:],
                            start=True,
                            stop=True,
                            tile_position=(0, 64),
                        )
                        x0a = p0.tile([128, NCH], f32, tag="x0a", name="x0a")
                        lrelu_from(p0, x0a[:], ps_a[:], ba_sb[:], [128, NCH])
                        x0b = p0.tile([128, NCH], f32, tag="x0b", name="x0b")
                        lrelu_from(p0, x0b[:], ps_b[:], bb_sb[:], [128, NCH])

                        for h, xdst in ((0, x1a), (1, x1b)):
                            ps_x = p0ps.tile(
                                [128, NCH], f32, tag="psx", name="ps_x"
                            )
                            nc.tensor.matmul(
                                ps_x[:],
                                wi_sb[:, 0, h * 128 : (h + 1) * 128],
                                x0a[:],
                                start=True,
                                stop=False,
                            )
                            nc.tensor.matmul(
                                ps_x[:],
                                wi_sb[:, 1, h * 128 : (h + 1) * 128],
                                x0b[:],
                                start=False,
                                stop=True,
                            )
                            lrelu_from(
                                p0,
                                xdst[:, c0 : c0 + NCH],
                                ps_x[:],
                                bi_sb[:, h : h + 1],
                                [128, NCH],
                            )

                # -------- h1 + AG1 --------
                def produce_h(xa, xb, w_sb, h_loc, pool, pps):
                    for r in range(2):
                        for t0 in range(0, T, 4):
                            nt = min(4, T - t0)
                            hsb = pool.tile(
                                [128, 4, 256], f16, tag="hsb", name="hsb"
                            )
                            for j in range(nt):
                                t = t0 + j
                                ts = slice(t * 128, (t + 1) * 128)
                                ph = pps.tile(
                                    [128, 256], f32, tag="ph", name="ph"
                                )
                                nc.tensor.matmul(
                                    ph[:],
                                    xa[:, ts],
                                    w_sb[:, 2 * r, :],
                                    start=True,
                                    stop=False,
                                )
                                nc.tensor.matmul(
                                    ph[:],
                                    xb[:, ts],
                                    w_sb[:, 2 * r + 1, :],
                                    start=False,
                                    stop=True,
                                )
                                nc.vector.tensor_copy(hsb[:, j, :], ph[:])
                            row0 = r * npad + t0 * 128
                            nc.sync.dma_start(
                                h_loc.ap()[row0 : row0 + nt * 128, :].rearrange(
                                    "(b p) f -> p b f", p=128
                                ),
                                hsb[:, 0:nt, :],
                            )

                with (
                    tc.tile_pool(name="hp", bufs=3) as hp,
                    tc.tile_pool(name="hpps", bufs=2, space="PSUM") as hpps,
                ):
                    produce_h(x1a, x1b, w1_sb, h1_loc, hp, hpps)
                AG(h1_loc, h1_full)

                # -------- RGCN layers (one pool set spans both) --------
                def rgcn_layer(
                    xa, xb, r_sb, b_sb, h_full, out_cb, lp, lps, rp_dram=None
                ):
                    use_rp = "rp" in variant
                    if use_rp:
                        # precompute root+bias for every tile into DRAM;
                        # this runs during the AllGather (no h dependency)
                        for t0 in range(0, T, 4):
                            nt = min(4, T - t0)
                            rst = lp.tile(
                                [128, 4, 256], f16, tag="rst", name="rst"
                            )
                            for j in range(nt):
                                t = t0 + j
                                ts = slice(t * 128, (t + 1) * 128)
                                po = lps.tile(
                                    [128, 256], f32, tag="po", name="po"
                                )
                                nc.tensor.matmul(
                                    po[:], xa[:, ts], r_sb[:, 0, :],
                                    start=True, stop=False,
                                )
                                nc.tensor.matmul(
                                    po[:], xb[:, ts], r_sb[:, 1, :],
                                    start=False, stop=True,
                                )
                                nc.vector.tensor_add(
                                    rst[:, j, :], po[:], b_sb[:]
                                )
                            nc.sync.dma_start(
                                rp_dram.ap()[
                                    t0 * 128 : (t0 + nt) * 128, :
                                ].rearrange("(b p) f -> p b f", p=128),
                                rst[:, 0:nt, :],
                            )
                    for t in range(T):
                        ts = slice(t * 128, (t + 1) * 128)
                        po = lps.tile(
                            [128, 256], f32, tag="po", name="po",
                            bufs=4 if "po4" in variant else 2,
                        )
                        nb_t = int(m.nb_tile[t])
                        if use_rp:
                            rre = lp.tile([128, 256], f16, tag="rre", name="rre")
                            nc.sync.dma_start(rre[:], rp_dram.ap()[ts, :])
                        else:
                            nc.tensor.matmul(
                                po[:], xa[:, ts], r_sb[:, 0, :],
                                start=True, stop=False,
                            )
                            nc.tensor.matmul(
                                po[:], xb[:, ts], r_sb[:, 1, :],
                                start=False, stop=(nb_t == 0),
                            )
                        ssb = lp.tile(
                            [128, int(m.nb_tile.max()), 128], f16,
                            tag="ssb", name="ssb",
                        )
                        nboff = int(m.tile_nboff[t])
                        for bi in range(nb_t):
                            nc.vector.tensor_scalar(
                                ssb[:, bi, :],
                                io_sb[:],
                                dst_sb[:, nboff + bi : nboff + bi + 1],
                                invc_sb[:, nboff + bi : nboff + bi + 1],
                                op0=mybir.AluOpType.is_equal,
                                op1=mybir.AluOpType.mult,
                            )
                        bi = 0
                        done = 0
                        for g in range(NGR):
                            B = int(m.bud[t, g])
                            if B == 0:
                                continue
                            gb = g * CHUNK
                            gs = m.chunk_sizes[g]
                            nbg = int(m.gnb[t, g])
                            msg = lp.tile(
                                [128, int(m.gnb[:, g].max()), 256],
                                f16,
                                tag=f"msg{g}",
                                name=f"msg{g}",
                                bufs=3 if "b3" in variant else 2,
                            )
                            o = int(m.goff[t, g])
                            if "seq_gather" in variant:
                                nc.sync.dma_start(
                                    msg[:, 0:nbg, :],
                                    h_full.ap()[gb : gb + nbg * 128, :].rearrange(
                                        "(b p) f -> p b f", p=128
                                    ),
                                )
                            else:
                                nc.gpsimd.dma_gather(
                                    msg[:, 0:nbg, :],
                                    h_full.ap()[gb : gb + gs, :],
                                    idx_sb[:, o // 16 : (o + B) // 16],
                                    num_idxs=B,
                                    num_idxs_reg=B,
                                    elem_size=256,
                                    single_packet="sp0" not in variant,
                                    queue_num=(g % nq),
                                )
                            for b in range(nbg):
                                done += 1
                                K = min(128, B - b * 128)
                                nc.tensor.matmul(
                                    po[:],
                                    ssb[0:K, bi, :],
                                    msg[0:K, b, :],
                                    start=(use_rp and done == 1),
                                    stop=(done == nb_t),
                                )
                                bi += 1
                        osb = lp.tile([128, 256], f32, tag="osb", name="osb")
                        if use_rp:
                            if nb_t > 0:
                                nc.vector.tensor_add(osb[:], po[:], rre[:])
                            else:
                                nc.vector.tensor_copy(osb[:], rre[:])
                        else:
                            nc.vector.tensor_add(osb[:], po[:], b_sb[:])
                        out_cb(t, ts, osb, lp, lps)

                h2_stage = [None, None]

                def l1_out(t, ts, osb, lp, lps):
                    for h, xdst in ((0, x2a), (1, x2b)):
                        pt = lps.tile([128, 128], f32, tag="pt", name="pt")
                        nc.tensor.transpose(
                            pt[:], osb[:, h * 128 : (h + 1) * 128], id_sb[:]
                        )
                        nc.vector.tensor_copy(xdst[:, ts], pt[:])
                    j = t % 4
                    if j == 0:
                        h2_stage[0] = lp.tile(
                            [128, 4, 256], f16, tag="h2s0", name="h2s0"
                        )
                        h2_stage[1] = lp.tile(
                            [128, 4, 256], f16, tag="h2s1", name="h2s1"
                        )
                    for r in range(2):
                        ph = lps.tile([128, 256], f32, tag="paux", name="ph2")
                        nc.tensor.matmul(
                            ph[:], x2a[:, ts], w2_sb[:, 2 * r, :],
                            start=True, stop=False,
                        )
                        nc.tensor.matmul(
                            ph[:], x2b[:, ts], w2_sb[:, 2 * r + 1, :],
                            start=False, stop=True,
                        )
                        nc.vector.tensor_copy(h2_stage[r][:, j, :], ph[:])
                    if j == 3 or t == T - 1:
                        t0 = t - j
                        nt = j + 1
                        for r in range(2):
                            row0 = r * npad + t0 * 128
                            nc.sync.dma_start(
                                h2_loc.ap()[
                                    row0 : row0 + nt * 128, :
                                ].rearrange("(b p) f -> p b f", p=128),
                                h2_stage[r][:, 0:nt, :],
                            )

                def l2_out(t, ts, osb, lp, lps):
                    o2t = lp.tile([128, 2, 128], f32, tag="o2t", name="o2t")
                    for h in range(2):
                        pt = lps.tile([128, 128], f32, tag="pt", name="pt")
                        nc.tensor.transpose(
                            pt[:], osb[:, h * 128 : (h + 1) * 128], id_sb[:]
                        )
                        nc.vector.tensor_copy(o2t[:, h, :], pt[:])
                    ht = lp.tile([128, 2, 128], f32, tag="ht", name="ht")
                    for h in range(2):
                        phd = lps.tile([128, 256], f32, tag="paux", name="phd")
                        nc.tensor.matmul(
                            phd[:, 0:128],
                            wo1_sb[:, 0, h * 128 : (h + 1) * 128],
                            o2t[:, 0, :],
                            start=True,
                            stop=False,
                        )
                        nc.tensor.matmul(
                            phd[:, 0:128],
                            wo1_sb[:, 1, h * 128 : (h + 1) * 128],
                            o2t[:, 1, :],
                            start=False,
                            stop=True,
                        )
                        lrelu_from(
                            lp, ht[:, h, :], phd[:, 0:128], bo1_sb[:, h : h + 1],
                            [128, 128],
                        )
                    pf = lps.tile([128, 256], f32, tag="paux", name="pf")
                    nc.tensor.matmul(
                        pf[:, 0:2], ht[:, 0, :], wo2_sb[:, 0, :],
                        start=True, stop=False,
                    )
                    nc.tensor.matmul(
                        pf[:, 0:2], ht[:, 1, :], wo2_sb[:, 1, :],
                        start=False, stop=True,
                    )
                    nc.vector.tensor_add(
                        out_stage[:, t, :], pf[:, 0:2], bo2_sb[:]
                    )
                    if t == T - 1:
                        nc.sync.dma_start(
                            out_ext.ap().rearrange("(b p) f -> p b f", p=128),
                            out_stage[:],
                        )

                with (
                    tc.tile_pool(name="lyr", bufs=2) as lp,
                    tc.tile_pool(name="lyrps", bufs=2, space="PSUM") as lps,
                ):
                    rgcn_layer(
                        x1a, x1b, r1_sb, b1_sb, h1_full, l1_out, lp, lps,
                        rp_dram=rp1,
                    )
                    AG(h2_loc, h2_full)
                    rgcn_layer(
                        x2a, x2b, r2_sb, b2_sb, h2_full, l2_out, lp, lps,
                        rp_dram=rp2,
                    )

            for _rep in range(repeats):
                whole_body()

    nc.compile()
    return nc


def _pmaj(t):
    """DRAM tensor [a, 128, b] viewed partition-major [128, a, b]."""
    return t.ap().rearrange("a p b -> p a b")


def build_null(m):
    """Same external I/O as build(), trivial body — launch-overhead probe."""
    nc = bacc.Bacc(
        "TRN2",
        target_bir_lowering=False,
        debug=False,
        enable_asserts=True,
        num_devices=m.ncores,
    )
    npad = m.npad
    f32, f16, i16 = dt.float32, dt.float16, dt.int16
    ein, eout = "ExternalInput", "ExternalOutput"
    nc.dram_tensor("desT", [6, 128, npad], f32, kind=ein)
    nc.dram_tensor("tweetT", [6, 128, npad], f32, kind=ein)
    nc.dram_tensor("numT", [5, npad], f32, kind=ein)
    nc.dram_tensor("catT", [3, npad], f32, kind=ein)
    nc.dram_tensor("idx", [128, m.totidx // 16], i16, kind=ein)
    nc.dram_tensor("dstb", [128, m.totnb], f32, kind=ein)
    nc.dram_tensor("invcb", [128, m.totnb], f32, kind=ein)
    nc.dram_tensor("iota", [128, 128], f16, kind=ein)
    for name, shape in [
        ("Wd6", [6, 128, 64]), ("Wt6", [6, 128, 64]), ("Wn", [5, 64]),
        ("Wc", [3, 64]), ("Wi2", [2, 128, 256]), ("W1", [2, 2, 128, 256]),
        ("root1", [2, 128, 256]), ("W2", [2, 2, 128, 256]),
        ("root2", [2, 128, 256]), ("Wo1", [2, 128, 256]), ("Wo2", [2, 128, 2]),
        ("bias_a", [128, 1]), ("bias_b", [128, 1]), ("bi_col", [128, 2]),
        ("bias1_rep", [128, 256]), ("bias2_rep", [128, 256]),
        ("bo1_col", [128, 2]), ("bo2_rep", [128, 2]), ("ident", [128, 128]),
    ]:
        nc.dram_tensor(name, shape, f32, kind=ein)
    out_ext = nc.dram_tensor("out", [npad, 2], f32, kind=eout)
    with tile.TileContext(nc) as tc:
        with tc.tile_pool(name="p", bufs=1) as p:
            t = p.tile([128, 2], f32)
            nc.gpsimd.memset(t[:], 0.0)
            for t0 in range(npad // 128):
                nc.sync.dma_start(
                    out_ext.ap()[t0 * 128 : (t0 + 1) * 128, :], t[:]
                )
    nc.compile()
    return nc


# ---------------------------------------------------------------- entry
def kernel(**inputs):
    meta, in_maps = prepare(inputs)
    nc = build(meta, variant=DEFAULT_VARIANT)
    from concourse.bass_utils import run_bass_kernel_spmd

    res = run_bass_kernel_spmd(
        nc, in_maps, core_ids=list(range(meta.ncores))
    ).results
    out = np.concatenate(
        [res[c]["out"][: meta.nsh] for c in range(meta.ncores)], axis=0
    )
    return out.astype(np.float32)

